# revision 8
# baseline (speedup 1.0000x reference)
"""Trainium2 Bass kernel for nn_MultiHeadContrastive (two-head contrastive loss).

Strategy (8 NeuronCores, two SPMD launches, no collectives):

  Launch 1 (MLP): rows of roi_feats are sorted by group
  (anchor / fg-low-iou / bg / ignore) on the host and sharded contiguously,
  1024 rows per core.  Each core computes both projection heads for its rows
  (transposed layout zT = [d, rows], fp32) via TensorE and returns the raw
  (pre-normalization) embeddings.

  Host: gathers the 8 z shards, L2-normalizes rows in float64, casts fp32.

  Launch 2 (SIM): every core receives the full normalized key matrices
  zT_fg [64, 8192], zT_cls [128, 8192] plus its private slice of anchor
  columns.  For each 128-anchor block it computes sim^T = anchors x keys via
  TensorE into PSUM (raw dot products), then ScalarE evaluates
  exp(dot / TAU) in place with accum_out producing per-anchor row sums per
  key range.  Because rows were sorted, the three masked sums the losses
  need (all keys / fg keys / non-ignored keys) are plain prefix-range sums,
  so no mask tensors and no second pass over the N^2 matrix exist at all.
  Anchors are restricted to rows with label>0, not ignored, and iou>0.5 —
  every other row contributes exactly zero to the weighted losses.

  Host: subtracts the self-similarity terms, computes the class-positive
  term of the SupCon loss from per-class sums of z (an O(N*D) computation),
  applies logs/weights in float64, and returns the 2-element loss vector.
"""

import math
import os

import numpy as np

import concourse.bass as bass
import concourse.bacc as bacc
import concourse.mybir as mybir
import concourse.tile as tile
from concourse.bass_utils import run_bass_kernel_spmd

N_CORES = 8
N, C = 8192, 1024
HID, DF, DC = 256, 64, 128
TAU = 0.2
EPS = 1e-8
EPS12 = 1e-12
IOU_THRESHOLD = 0.5

F32 = mybir.dt.float32
ACT = mybir.ActivationFunctionType
AX = mybir.AxisListType

# Introspection for test.py: BassKernelResults of the two launches.
LAST_RESULTS = []


def _build_mlp_nc():
    """Launch 1: per-core MLP producing raw zT for both heads."""
    R = N // N_CORES  # rows per core
    KC = C // 128     # feature chunks
    KH = HID // 128   # hidden chunks
    RB = 512          # moving free dim per matmul (fp32 limit)
    NR = R // RB

    nc = bacc.Bacc(trn_type="TRN2", num_devices=N_CORES, debug=False)
    xT = nc.dram_tensor("xT", [C, R], F32, kind="ExternalInput")
    w1fT = nc.dram_tensor("w1fT", [C, HID], F32, kind="ExternalInput")
    w2fT = nc.dram_tensor("w2fT", [HID, DF], F32, kind="ExternalInput")
    w1cT = nc.dram_tensor("w1cT", [C, HID], F32, kind="ExternalInput")
    w2cT = nc.dram_tensor("w2cT", [HID, DC], F32, kind="ExternalInput")
    b1f = nc.dram_tensor("b1f", [HID, 1], F32, kind="ExternalInput")
    b2f = nc.dram_tensor("b2f", [DF, 1], F32, kind="ExternalInput")
    b1c = nc.dram_tensor("b1c", [HID, 1], F32, kind="ExternalInput")
    b2c = nc.dram_tensor("b2c", [DC, 1], F32, kind="ExternalInput")
    zf = nc.dram_tensor("zf", [DF, R], F32, kind="ExternalOutput")
    zc = nc.dram_tensor("zc", [DC, R], F32, kind="ExternalOutput")

    with tile.TileContext(nc) as tc:
        with (
            tc.tile_pool(name="cst", bufs=1) as cst,
            tc.tile_pool(name="hb", bufs=2) as hb,
            tc.tile_pool(name="zb", bufs=2) as zb,
            tc.tile_pool(name="ps", bufs=3, space="PSUM") as ps,
        ):
            xt = cst.tile([128, KC, R], F32, tag="xt")
            for k in range(KC):
                nc.sync.dma_start(out=xt[:, k, :], in_=xT[k * 128:(k + 1) * 128, :])

            for hname, w1d, w2d, b1d, b2d, d, zout in (
                ("f", w1fT, w2fT, b1f, b2f, DF, zf),
                ("c", w1cT, w2cT, b1c, b2c, DC, zc),
            ):
                w1t = cst.tile([128, KC, HID], F32, tag=f"w1{hname}")
                for k in range(KC):
                    nc.sync.dma_start(out=w1t[:, k, :], in_=w1d[k * 128:(k + 1) * 128, :])
                w2t = cst.tile([128, KH, d], F32, tag=f"w2{hname}")
                for h in range(KH):
                    nc.sync.dma_start(out=w2t[:, h, :], in_=w2d[h * 128:(h + 1) * 128, :])
                b1t = cst.tile([128, KH], F32, tag=f"b1{hname}")
                for h in range(KH):
                    nc.sync.dma_start(out=b1t[:, h:h + 1], in_=b1d[h * 128:(h + 1) * 128, :])
                b2t = cst.tile([d, 1], F32, tag=f"b2{hname}")
                nc.sync.dma_start(out=b2t[:, :], in_=b2d[:, :])

                hsb = hb.tile([128, KH, R], F32, tag=f"h{hname}")
                for r in range(NR):
                    for h in range(KH):
                        hp = ps.tile([128, RB], F32, tag="hp")
                        for k in range(KC):
                            nc.tensor.matmul(
                                out=hp[:, :],
                                lhsT=w1t[:, k, h * 128:(h + 1) * 128],
                                rhs=xt[:, k, r * RB:(r + 1) * RB],
                                start=(k == 0),
                                stop=(k == KC - 1),
                            )
                        # hT = relu(w1 @ xT + b1); b1 is per-partition here.
                        nc.scalar.activation(
                            out=hsb[:, h, r * RB:(r + 1) * RB],
                            in_=hp[:, :],
                            func=ACT.Relu,
                            bias=b1t[:, h:h + 1],
                            scale=1.0,
                        )
                    zp = ps.tile([128, RB], F32, tag="zp")
                    for h in range(KH):
                        nc.tensor.matmul(
                            out=zp[:d, :],
                            lhsT=w2t[:, h, :],
                            rhs=hsb[:, h, r * RB:(r + 1) * RB],
                            start=(h == 0),
                            stop=(h == KH - 1),
                        )
                    zt = zb.tile([d, RB], F32, tag=f"z{hname}")
                    nc.scalar.activation(
                        out=zt[:, :],
                        in_=zp[:d, :],
                        func=ACT.Identity,
                        bias=b2t[:, 0:1],
                        scale=1.0,
                    )
                    nc.sync.dma_start(out=zout[:, r * RB:(r + 1) * RB], in_=zt[:, :])
    nc.compile()
    return nc


def _build_sim_nc(n_fg, n_valid, nblk):
    """Launch 2: per-anchor-block sim matmuls + fused exp/prefix-range sums.

    Returns (nc, numer_cols, nfgcols, ngc): stats output columns are
      0: sum_{all keys} exp(sim/TAU)
      1: sum_{keys < n_fg} exp(sim/TAU)
      2: sum_{keys < n_valid} exp(sim/TAU)
    (all including the anchor's self term, subtracted on the host).
    """
    A = nblk * 128
    G = 2048
    NGF = N // G
    NGC = (n_valid + G - 1) // G

    nc = bacc.Bacc(trn_type="TRN2", num_devices=N_CORES, debug=False)
    zfk = nc.dram_tensor("zfk", [DF, N], F32, kind="ExternalInput")
    zck = nc.dram_tensor("zck", [DC, N], F32, kind="ExternalInput")
    zfa = nc.dram_tensor("zfa", [DF, A], F32, kind="ExternalInput")
    zca = nc.dram_tensor("zca", [DC, A], F32, kind="ExternalInput")
    stats = nc.dram_tensor("stats", [nblk, 128, 3], F32, kind="ExternalOutput")

    # fg-head exp/accum pieces: split each 2048-key group at the n_fg
    # boundary so masked sums become plain column-range selections.
    fg_pieces = []  # (group, c0, c1, col)
    col = 0
    numer_cols = 0
    for g in range(NGF):
        lo, hi = g * G, (g + 1) * G
        cuts = [lo, n_fg, hi] if lo < n_fg < hi else [lo, hi]
        for a0, a1 in zip(cuts[:-1], cuts[1:]):
            fg_pieces.append((g, a0 - lo, a1 - lo, col))
            if a1 <= n_fg:
                numer_cols = col + 1
            col += 1
    nfgcols = col

    with tile.TileContext(nc) as tc:
        with (
            tc.tile_pool(name="keys", bufs=1) as keys,
            tc.tile_pool(name="anch", bufs=1) as anch,
            tc.tile_pool(name="st", bufs=3) as st,
            tc.tile_pool(name="ps", bufs=2, space="PSUM") as ps,
        ):
            zfk_t = []
            for g in range(NGF):
                t = keys.tile([DF, G], F32, tag=f"zfk{g}")
                nc.sync.dma_start(out=t[:, :], in_=zfk[:, g * G:(g + 1) * G])
                zfk_t.append(t)
            zck_t = []
            for g in range(NGC):
                klim = min(G, n_valid - g * G)
                t = keys.tile([DC, G], F32, tag=f"zck{g}")
                nc.sync.dma_start(out=t[:, 0:klim], in_=zck[:, g * G:g * G + klim])
                zck_t.append(t)
            zfa_t = anch.tile([DF, A], F32, tag="zfa")
            nc.sync.dma_start(out=zfa_t[:, :], in_=zfa[:, :])
            zca_t = anch.tile([DC, A], F32, tag="zca")
            nc.sync.dma_start(out=zca_t[:, :], in_=zca[:, :])

            for ab in range(nblk):
                lf = zfa_t[:, ab * 128:(ab + 1) * 128]
                lc = zca_t[:, ab * 128:(ab + 1) * 128]
                sf = st.tile([128, nfgcols], F32, tag="sf")
                sc = st.tile([128, NGC], F32, tag="sc")
                for g in range(NGF):
                    p = ps.tile([128, G], F32, tag="ps")
                    for kk in range(G // 512):
                        nc.tensor.matmul(
                            out=p[:, kk * 512:(kk + 1) * 512],
                            lhsT=lf,
                            rhs=zfk_t[g][:, kk * 512:(kk + 1) * 512],
                            start=True,
                            stop=True,
                        )
                    for gg, c0, c1, pcol in fg_pieces:
                        if gg != g:
                            continue
                        nc.scalar.activation(
                            out=p[:, c0:c1],
                            in_=p[:, c0:c1],
                            func=ACT.Exp,
                            scale=1.0 / TAU,
                            accum_out=sf[:, pcol:pcol + 1],
                        )
                for g in range(NGC):
                    klim = min(G, n_valid - g * G)
                    p = ps.tile([128, G], F32, tag="ps")
                    for kk in range((klim + 511) // 512):
                        w = min(512, klim - kk * 512)
                        nc.tensor.matmul(
                            out=p[:, kk * 512:kk * 512 + w],
                            lhsT=lc,
                            rhs=zck_t[g][:, kk * 512:kk * 512 + w],
                            start=True,
                            stop=True,
                        )
                    nc.scalar.activation(
                        out=p[:, 0:klim],
                        in_=p[:, 0:klim],
                        func=ACT.Exp,
                        scale=1.0 / TAU,
                        accum_out=sc[:, g:g + 1],
                    )
                o3 = st.tile([128, 3], F32, tag="o3")
                nc.vector.reduce_sum(out=o3[:, 0:1], in_=sf[:, 0:nfgcols], axis=AX.X)
                nc.vector.reduce_sum(out=o3[:, 1:2], in_=sf[:, 0:numer_cols], axis=AX.X)
                nc.vector.reduce_sum(out=o3[:, 2:3], in_=sc[:, 0:NGC], axis=AX.X)
                nc.sync.dma_start(out=stats[ab, :, :], in_=o3[:, :])
    nc.compile()
    return nc


LAST_TIMES = []


def _run(nc, in_maps, out_names):
    import time as _time

    if os.environ.get("CC_BASS_SIM") == "1":
        from concourse import bass_interp

        results = []
        for m in range(N_CORES):
            sim = bass_interp.CoreSim(nc, core_id=m)
            for k, v in in_maps[m].items():
                sim.tensor(k)[:] = v
            if nc.partition_id_tensor is not None:
                sim.tensor(nc.partition_id_tensor.name)[:] = np.array(
                    [[m]], dtype=np.uint32
                )
            sim.simulate()
            results.append(
                {name: np.array(sim.mem_tensor(name)) for name in out_names}
            )
        return results
    t0 = _time.monotonic()
    res = run_bass_kernel_spmd(nc, in_maps, core_ids=list(range(N_CORES)))
    LAST_TIMES.append(_time.monotonic() - t0)
    LAST_RESULTS.append(res)
    return res.results


def kernel(**inputs):
    global LAST_RESULTS, LAST_TIMES
    LAST_RESULTS = []
    LAST_TIMES = []

    roi = np.ascontiguousarray(np.asarray(inputs["roi_feats"], dtype=np.float32))
    labels = np.asarray(inputs["labels"]).astype(np.int64)
    ious = np.asarray(inputs["ious"], dtype=np.float32)
    w1f = np.asarray(inputs["w1f"], dtype=np.float32)
    b1f = np.asarray(inputs["b1f"], dtype=np.float32)
    w2f = np.asarray(inputs["w2f"], dtype=np.float32)
    b2f = np.asarray(inputs["b2f"], dtype=np.float32)
    w1c = np.asarray(inputs["w1c"], dtype=np.float32)
    b1c = np.asarray(inputs["b1c"], dtype=np.float32)
    w2c = np.asarray(inputs["w2c"], dtype=np.float32)
    b2c = np.asarray(inputs["b2c"], dtype=np.float32)
    assert roi.shape == (N, C)

    ign = labels == -1
    fg = (labels > 0) & ~ign
    bg = (labels == 0) & ~ign
    anc = fg & (ious > IOU_THRESHOLD)

    perm = np.concatenate(
        [
            np.where(anc)[0],
            np.where(fg & ~anc)[0],
            np.where(bg)[0],
            np.where(ign)[0],
        ]
    )
    n_A = int(anc.sum())
    n_fg = int(fg.sum())
    n_valid = n_fg + int(bg.sum())

    if n_A == 0:
        return np.zeros(2, dtype=np.float32)

    x_s = roi[perm]
    labels_s = labels[perm]
    ious_s = ious[perm].astype(np.float64)

    # ---------------- launch 1: MLP ----------------
    nc1 = _build_mlp_nc()
    xT = np.ascontiguousarray(x_s.T)  # [C, N]
    R = N // N_CORES
    shared1 = {
        "w1fT": np.ascontiguousarray(w1f.T),
        "w2fT": np.ascontiguousarray(w2f.T),
        "w1cT": np.ascontiguousarray(w1c.T),
        "w2cT": np.ascontiguousarray(w2c.T),
        "b1f": b1f.reshape(HID, 1).copy(),
        "b2f": b2f.reshape(DF, 1).copy(),
        "b1c": b1c.reshape(HID, 1).copy(),
        "b2c": b2c.reshape(DC, 1).copy(),
    }
    in_maps1 = [
        {"xT": np.ascontiguousarray(xT[:, m * R:(m + 1) * R]), **shared1}
        for m in range(N_CORES)
    ]
    res1 = _run(nc1, in_maps1, ["zf", "zc"])

    zfT_raw = np.concatenate([r["zf"] for r in res1], axis=1)  # [DF, N]
    zcT_raw = np.concatenate([r["zc"] for r in res1], axis=1)  # [DC, N]

    # ---------------- host: normalize in float64, cast fp32 ----------------
    def _normalize(zT_raw):
        z = zT_raw.T.astype(np.float64)  # [N, d]
        nrm = np.sqrt(np.sum(z * z, axis=1, keepdims=True))
        zn = z / np.maximum(nrm, EPS)
        return zn.astype(np.float32)

    zfn = _normalize(zfT_raw)  # [N, DF] fp32, sorted order
    zcn = _normalize(zcT_raw)  # [N, DC]

    # ---------------- launch 2: sims ----------------
    nblk = max(1, math.ceil(math.ceil(n_A / N_CORES) / 128))
    A_pc = nblk * 128
    nc2 = _build_sim_nc(n_fg, n_valid, nblk)

    zfkT = np.ascontiguousarray(zfn.T)  # [DF, N]
    zckT = np.ascontiguousarray(zcn.T)  # [DC, N]
    in_maps2 = []
    for m in range(N_CORES):
        idx = np.minimum(np.arange(m * A_pc, (m + 1) * A_pc), n_A - 1)
        in_maps2.append(
            {
                "zfk": zfkT,
                "zck": zckT,
                "zfa": np.ascontiguousarray(zfkT[:, idx]),
                "zca": np.ascontiguousarray(zckT[:, idx]),
            }
        )
    res2 = _run(nc2, in_maps2, ["stats"])

    # slot s of the concatenated stats covers anchor s; drop padded slots
    stats = np.concatenate([r["stats"].reshape(A_pc, 3) for r in res2], axis=0)
    stats = stats[np.arange(N_CORES * A_pc) < n_A].astype(np.float64)  # [n_A, 3]

    # ---------------- host: final losses in float64 ----------------
    zfa64 = zfn[:n_A].astype(np.float64)
    zca64 = zcn[:n_A].astype(np.float64)
    w_a = ious_s[:n_A]

    selfdot_f = np.sum(zfa64 * zfa64, axis=1)
    selfexp_f = np.exp(selfdot_f / TAU)
    selfdot_c = np.sum(zca64 * zca64, axis=1)
    selfexp_c = np.exp(selfdot_c / TAU)

    # fg/bg loss
    npos_fg = n_fg - 1
    if npos_fg > 0:
        denom = stats[:, 0] - selfexp_f
        numer = stats[:, 1] - selfexp_f
        li = -np.log((numer + EPS) / (denom + EPS))
        loss_fg = np.sum(li * w_a) / (np.sum(w_a) + EPS)
    else:
        loss_fg = 0.0  # num=0, den=EPS -> 0

    # class supcon loss
    lab_valid = labels_s[:n_valid]
    cnt = np.bincount(lab_valid, minlength=21)
    S = np.zeros((21, DC), dtype=np.float64)
    np.add.at(S, lab_valid, zcn[:n_valid].astype(np.float64))
    c_a = labels_s[:n_A]
    n_pos = (cnt[c_a] - 1).astype(np.float64)
    D = stats[:, 2] - selfexp_c
    denom_log = np.log(np.maximum(D, 1e-300))
    sum_pos = (np.einsum("nd,nd->n", zca64, S[c_a]) - selfdot_c) / TAU
    li_c = -(sum_pos - n_pos * denom_log) / np.maximum(n_pos, 1.0)
    valid_c = n_pos > 0
    num2 = np.sum(np.where(valid_c, li_c * w_a, 0.0))
    den2 = np.sum(np.where(valid_c, w_a, 0.0))
    loss_cls = num2 / (den2 + EPS12)

    return np.stack([loss_fg, loss_cls]).astype(np.float32)


# revision 15
# speedup vs baseline: 1.0521x; 1.0521x over previous
"""Trainium2 Bass kernel for nn_MultiHeadContrastive (two-head contrastive loss).

Strategy (8 NeuronCores, two SPMD launches, no collectives):

  Launch 1 (MLP): rows of roi_feats are sorted by group
  (anchor / fg-low-iou / bg / ignore) on the host and sharded contiguously,
  1024 rows per core.  Each core computes both projection heads for its rows
  (transposed layout zT = [d, rows], fp32) via TensorE and returns the raw
  (pre-normalization) embeddings.

  Host: gathers the 8 z shards, L2-normalizes rows in float64, casts fp32.

  Launch 2 (SIM): every core receives the full normalized key matrices
  zT_fg [64, 8192], zT_cls [128, 8192] plus its private slice of anchor
  columns.  For each 128-anchor block it computes sim^T = anchors x keys via
  TensorE into PSUM (raw dot products), then ScalarE evaluates
  exp(dot / TAU) in place with accum_out producing per-anchor row sums per
  key range.  Because rows were sorted, the three masked sums the losses
  need (all keys / fg keys / non-ignored keys) are plain prefix-range sums,
  so no mask tensors and no second pass over the N^2 matrix exist at all.
  Anchors are restricted to rows with label>0, not ignored, and iou>0.5 —
  every other row contributes exactly zero to the weighted losses.

  Host: subtracts the self-similarity terms, computes the class-positive
  term of the SupCon loss from per-class sums of z (an O(N*D) computation),
  applies logs/weights in float64, and returns the 2-element loss vector.
"""

import math
import os

import numpy as np

import concourse.bass as bass
import concourse.bacc as bacc
import concourse.mybir as mybir
import concourse.tile as tile
from concourse.bass_utils import run_bass_kernel_spmd

N_CORES = 8
N, C = 8192, 1024
HID, DF, DC = 256, 64, 128
TAU = 0.2
EPS = 1e-8
EPS12 = 1e-12
IOU_THRESHOLD = 0.5

F32 = mybir.dt.float32
F32R = mybir.dt.float32r
ACT = mybir.ActivationFunctionType
AX = mybir.AxisListType

# Introspection for test.py: BassKernelResults of the two launches.
LAST_RESULTS = []


def _build_mlp_nc():
    """Launch 1: per-core MLP producing raw zT for both heads."""
    R = N // N_CORES  # rows per core
    KC = C // 128     # feature chunks
    KH = HID // 128   # hidden chunks
    RB = 512          # moving free dim per matmul (fp32 limit)
    NR = R // RB

    nc = bacc.Bacc(trn_type="TRN2", num_devices=N_CORES, debug=False)
    xT = nc.dram_tensor("xT", [C, R], F32R, kind="ExternalInput")
    w1fT = nc.dram_tensor("w1fT", [C, HID], F32R, kind="ExternalInput")
    w2fT = nc.dram_tensor("w2fT", [HID, DF], F32R, kind="ExternalInput")
    w1cT = nc.dram_tensor("w1cT", [C, HID], F32R, kind="ExternalInput")
    w2cT = nc.dram_tensor("w2cT", [HID, DC], F32R, kind="ExternalInput")
    b1f = nc.dram_tensor("b1f", [HID, 1], F32, kind="ExternalInput")
    b2f = nc.dram_tensor("b2f", [DF, 1], F32, kind="ExternalInput")
    b1c = nc.dram_tensor("b1c", [HID, 1], F32, kind="ExternalInput")
    b2c = nc.dram_tensor("b2c", [DC, 1], F32, kind="ExternalInput")
    zf = nc.dram_tensor("zf", [DF, R], F32, kind="ExternalOutput")
    zc = nc.dram_tensor("zc", [DC, R], F32, kind="ExternalOutput")

    with tile.TileContext(nc) as tc:
        with (
            tc.tile_pool(name="cst", bufs=1) as cst,
            tc.tile_pool(name="hb", bufs=2) as hb,
            tc.tile_pool(name="zb", bufs=2) as zb,
            tc.tile_pool(name="ps", bufs=3, space="PSUM") as ps,
        ):
            # per-chunk tiles so matmuls on chunk k only wait for chunk k's DMA
            xt_t = []
            for k in range(KC):
                t = cst.tile([128, R], F32R, tag=f"xt{k}")
                nc.sync.dma_start(out=t[:, :], in_=xT[k * 128:(k + 1) * 128, :])
                xt_t.append(t)

            for hname, w1d, w2d, b1d, b2d, d, zout in (
                ("f", w1fT, w2fT, b1f, b2f, DF, zf),
                ("c", w1cT, w2cT, b1c, b2c, DC, zc),
            ):
                w1_t = []
                for k in range(KC):
                    t = cst.tile([128, HID], F32R, tag=f"w1{hname}{k}")
                    nc.sync.dma_start(out=t[:, :], in_=w1d[k * 128:(k + 1) * 128, :])
                    w1_t.append(t)
                w2t = cst.tile([128, KH, d], F32R, tag=f"w2{hname}")
                for h in range(KH):
                    nc.sync.dma_start(out=w2t[:, h, :], in_=w2d[h * 128:(h + 1) * 128, :])
                b1t = cst.tile([128, KH], F32, tag=f"b1{hname}")
                for h in range(KH):
                    nc.sync.dma_start(out=b1t[:, h:h + 1], in_=b1d[h * 128:(h + 1) * 128, :])
                b2t = cst.tile([d, 1], F32, tag=f"b2{hname}")
                nc.sync.dma_start(out=b2t[:, :], in_=b2d[:, :])

                hsb = hb.tile([128, KH, R], F32R, tag=f"h{hname}")
                for r in range(NR):
                    for h in range(KH):
                        hp = ps.tile([128, RB], F32, tag="hp")
                        for k in range(KC):
                            nc.tensor.matmul(
                                out=hp[:, :],
                                lhsT=w1_t[k][:, h * 128:(h + 1) * 128],
                                rhs=xt_t[k][:, r * RB:(r + 1) * RB],
                                start=(k == 0),
                                stop=(k == KC - 1),
                            )
                        # hT = relu(w1 @ xT + b1) fused on DVE (also rounds
                        # to fp32r for the next matmul); b1 is per-partition.
                        nc.vector.tensor_scalar(
                            out=hsb[:, h, r * RB:(r + 1) * RB],
                            in0=hp[:, :],
                            scalar1=b1t[:, h:h + 1],
                            scalar2=0.0,
                            op0=mybir.AluOpType.add,
                            op1=mybir.AluOpType.max,
                        )
                    zp = ps.tile([128, RB], F32, tag="zp")
                    for h in range(KH):
                        nc.tensor.matmul(
                            out=zp[:d, :],
                            lhsT=w2t[:, h, :],
                            rhs=hsb[:, h, r * RB:(r + 1) * RB],
                            start=(h == 0),
                            stop=(h == KH - 1),
                        )
                    zt = zb.tile([d, RB], F32, tag=f"z{hname}")
                    nc.scalar.activation(
                        out=zt[:, :],
                        in_=zp[:d, :],
                        func=ACT.Identity,
                        bias=b2t[:, 0:1],
                        scale=1.0,
                    )
                    nc.sync.dma_start(out=zout[:, r * RB:(r + 1) * RB], in_=zt[:, :])
    nc.compile()
    return nc


def _build_sim_nc(n_fg, n_valid, nblk):
    """Launch 2: per-anchor-block sim matmuls + fused exp/prefix-range sums.

    Returns (nc, numer_cols, nfgcols, ngc): stats output columns are
      0: sum_{all keys} exp(sim/TAU)
      1: sum_{keys < n_fg} exp(sim/TAU)
      2: sum_{keys < n_valid} exp(sim/TAU)
    (all including the anchor's self term, subtracted on the host).
    """
    A = nblk * 128
    G = 2048
    NGF = N // G
    NGC = (n_valid + G - 1) // G

    nc = bacc.Bacc(trn_type="TRN2", num_devices=N_CORES, debug=False)
    zfk = nc.dram_tensor("zfk", [DF, N], F32R, kind="ExternalInput")
    zck = nc.dram_tensor("zck", [DC, N], F32R, kind="ExternalInput")
    zfa = nc.dram_tensor("zfa", [DF, A], F32R, kind="ExternalInput")
    zca = nc.dram_tensor("zca", [DC, A], F32R, kind="ExternalInput")
    stats = nc.dram_tensor("stats", [nblk, 128, 3], F32, kind="ExternalOutput")

    # fg-head exp/accum pieces: split each 2048-key group at the n_fg
    # boundary so masked sums become plain column-range selections.
    fg_pieces = []  # (group, c0, c1, col)
    col = 0
    numer_cols = 0
    for g in range(NGF):
        lo, hi = g * G, (g + 1) * G
        cuts = [lo, n_fg, hi] if lo < n_fg < hi else [lo, hi]
        for a0, a1 in zip(cuts[:-1], cuts[1:]):
            fg_pieces.append((g, a0 - lo, a1 - lo, col))
            if a1 <= n_fg:
                numer_cols = col + 1
            col += 1
    nfgcols = col

    with tile.TileContext(nc) as tc:
        with (
            tc.tile_pool(name="keys", bufs=1) as keys,
            tc.tile_pool(name="anch", bufs=1) as anch,
            tc.tile_pool(name="st", bufs=3) as st,
            tc.tile_pool(name="ps", bufs=2, space="PSUM") as ps,
        ):
            # anchors first: the very first matmul needs them
            zfa_t = anch.tile([DF, A], F32R, tag="zfa")
            nc.sync.dma_start(out=zfa_t[:, :], in_=zfa[:, :])
            zca_t = anch.tile([DC, A], F32R, tag="zca")
            nc.sync.dma_start(out=zca_t[:, :], in_=zca[:, :])
            # warm up the ACT exp table load while DMAs stream
            wu = st.tile([1, 8], F32, tag="wu")
            nc.vector.memset(wu[:, :], 0.0)
            nc.scalar.activation(out=wu[:, :], in_=wu[:, :], func=ACT.Exp, scale=1.0)
            zfk_t = []
            for g in range(NGF):
                t = keys.tile([DF, G], F32R, tag=f"zfk{g}")
                nc.sync.dma_start(out=t[:, :], in_=zfk[:, g * G:(g + 1) * G])
                zfk_t.append(t)
            zck_t = []
            for g in range(NGC):
                # load the full group (cols past n_valid are real rows too);
                # only the exp/accum below is range-restricted
                t = keys.tile([DC, G], F32R, tag=f"zck{g}")
                nc.sync.dma_start(out=t[:, :], in_=zck[:, g * G:(g + 1) * G])
                zck_t.append(t)

            for ab in range(nblk):
                lf = zfa_t[:, ab * 128:(ab + 1) * 128]
                lc = zca_t[:, ab * 128:(ab + 1) * 128]
                sf = st.tile([128, nfgcols], F32, tag="sf")
                sc = st.tile([128, NGC], F32, tag="sc")
                for g in range(NGF):
                    p = ps.tile([128, G], F32, tag="ps")
                    for kk in range(G // 512):
                        nc.tensor.matmul(
                            out=p[:, kk * 512:(kk + 1) * 512],
                            lhsT=lf,
                            rhs=zfk_t[g][:, kk * 512:(kk + 1) * 512],
                            start=True,
                            stop=True,
                        )
                    for gg, c0, c1, pcol in fg_pieces:
                        if gg != g:
                            continue
                        nc.scalar.activation(
                            out=p[:, c0:c1],
                            in_=p[:, c0:c1],
                            func=ACT.Exp,
                            scale=1.0 / TAU,
                            accum_out=sf[:, pcol:pcol + 1],
                        )
                for g in range(NGC):
                    klim = min(G, n_valid - g * G)
                    p = ps.tile([128, G], F32, tag="ps")
                    # full-width matmuls (fp32r needs large even free dims);
                    # only [0:klim] is exp'd/accumulated below
                    for kk in range(G // 512):
                        if kk * 512 >= klim:
                            break
                        nc.tensor.matmul(
                            out=p[:, kk * 512:(kk + 1) * 512],
                            lhsT=lc,
                            rhs=zck_t[g][:, kk * 512:(kk + 1) * 512],
                            start=True,
                            stop=True,
                        )
                    nc.scalar.activation(
                        out=p[:, 0:klim],
                        in_=p[:, 0:klim],
                        func=ACT.Exp,
                        scale=1.0 / TAU,
                        accum_out=sc[:, g:g + 1],
                    )
                o3 = st.tile([128, 3], F32, tag="o3")
                nc.vector.reduce_sum(out=o3[:, 0:1], in_=sf[:, 0:nfgcols], axis=AX.X)
                nc.vector.reduce_sum(out=o3[:, 1:2], in_=sf[:, 0:numer_cols], axis=AX.X)
                nc.vector.reduce_sum(out=o3[:, 2:3], in_=sc[:, 0:NGC], axis=AX.X)
                nc.sync.dma_start(out=stats[ab, :, :], in_=o3[:, :])
    nc.compile()
    return nc


LAST_TIMES = []


def _run(nc, in_maps, out_names):
    import time as _time

    if os.environ.get("CC_BASS_SIM") == "1":
        from concourse import bass_interp

        results = []
        for m in range(N_CORES):
            sim = bass_interp.CoreSim(nc, core_id=m)
            for k, v in in_maps[m].items():
                sim.tensor(k)[:] = v
            if nc.partition_id_tensor is not None:
                sim.tensor(nc.partition_id_tensor.name)[:] = np.array(
                    [[m]], dtype=np.uint32
                )
            sim.simulate()
            results.append(
                {name: np.array(sim.mem_tensor(name)) for name in out_names}
            )
        return results
    t0 = _time.monotonic()
    res = run_bass_kernel_spmd(nc, in_maps, core_ids=list(range(N_CORES)))
    LAST_TIMES.append(_time.monotonic() - t0)
    LAST_RESULTS.append(res)
    return res.results


def kernel(**inputs):
    global LAST_RESULTS, LAST_TIMES
    LAST_RESULTS = []
    LAST_TIMES = []

    roi = np.ascontiguousarray(np.asarray(inputs["roi_feats"], dtype=np.float32))
    labels = np.asarray(inputs["labels"]).astype(np.int64)
    ious = np.asarray(inputs["ious"], dtype=np.float32)
    w1f = np.asarray(inputs["w1f"], dtype=np.float32)
    b1f = np.asarray(inputs["b1f"], dtype=np.float32)
    w2f = np.asarray(inputs["w2f"], dtype=np.float32)
    b2f = np.asarray(inputs["b2f"], dtype=np.float32)
    w1c = np.asarray(inputs["w1c"], dtype=np.float32)
    b1c = np.asarray(inputs["b1c"], dtype=np.float32)
    w2c = np.asarray(inputs["w2c"], dtype=np.float32)
    b2c = np.asarray(inputs["b2c"], dtype=np.float32)
    assert roi.shape == (N, C)

    ign = labels == -1
    fg = (labels > 0) & ~ign
    bg = (labels == 0) & ~ign
    anc = fg & (ious > IOU_THRESHOLD)

    perm = np.concatenate(
        [
            np.where(anc)[0],
            np.where(fg & ~anc)[0],
            np.where(bg)[0],
            np.where(ign)[0],
        ]
    )
    n_A = int(anc.sum())
    n_fg = int(fg.sum())
    n_valid = n_fg + int(bg.sum())

    if n_A == 0:
        return np.zeros(2, dtype=np.float32)

    x_s = roi[perm]
    labels_s = labels[perm]
    ious_s = ious[perm].astype(np.float64)

    # ---------------- launch 1: MLP ----------------
    nc1 = _build_mlp_nc()
    xT = np.ascontiguousarray(x_s.T)  # [C, N]
    R = N // N_CORES
    shared1 = {
        "w1fT": np.ascontiguousarray(w1f.T),
        "w2fT": np.ascontiguousarray(w2f.T),
        "w1cT": np.ascontiguousarray(w1c.T),
        "w2cT": np.ascontiguousarray(w2c.T),
        "b1f": b1f.reshape(HID, 1).copy(),
        "b2f": b2f.reshape(DF, 1).copy(),
        "b1c": b1c.reshape(HID, 1).copy(),
        "b2c": b2c.reshape(DC, 1).copy(),
    }
    in_maps1 = [
        {"xT": np.ascontiguousarray(xT[:, m * R:(m + 1) * R]), **shared1}
        for m in range(N_CORES)
    ]
    res1 = _run(nc1, in_maps1, ["zf", "zc"])

    zfT_raw = np.concatenate([r["zf"] for r in res1], axis=1)  # [DF, N]
    zcT_raw = np.concatenate([r["zc"] for r in res1], axis=1)  # [DC, N]

    # ---------------- host: normalize in float64, cast fp32 ----------------
    def _normalize(zT_raw):
        z = zT_raw.T.astype(np.float64)  # [N, d]
        nrm = np.sqrt(np.sum(z * z, axis=1, keepdims=True))
        zn = z / np.maximum(nrm, EPS)
        return zn.astype(np.float32)

    zfn = _normalize(zfT_raw)  # [N, DF] fp32, sorted order
    zcn = _normalize(zcT_raw)  # [N, DC]

    # ---------------- launch 2: sims ----------------
    nblk = max(1, math.ceil(math.ceil(n_A / N_CORES) / 128))
    A_pc = nblk * 128
    nc2 = _build_sim_nc(n_fg, n_valid, nblk)

    zfkT = np.ascontiguousarray(zfn.T)  # [DF, N]
    zckT = np.ascontiguousarray(zcn.T)  # [DC, N]
    in_maps2 = []
    for m in range(N_CORES):
        idx = np.minimum(np.arange(m * A_pc, (m + 1) * A_pc), n_A - 1)
        in_maps2.append(
            {
                "zfk": zfkT,
                "zck": zckT,
                "zfa": np.ascontiguousarray(zfkT[:, idx]),
                "zca": np.ascontiguousarray(zckT[:, idx]),
            }
        )
    res2 = _run(nc2, in_maps2, ["stats"])

    # slot s of the concatenated stats covers anchor s; drop padded slots
    stats = np.concatenate([r["stats"].reshape(A_pc, 3) for r in res2], axis=0)
    stats = stats[np.arange(N_CORES * A_pc) < n_A].astype(np.float64)  # [n_A, 3]

    # ---------------- host: final losses in float64 ----------------
    zfa64 = zfn[:n_A].astype(np.float64)
    zca64 = zcn[:n_A].astype(np.float64)
    w_a = ious_s[:n_A]

    selfdot_f = np.sum(zfa64 * zfa64, axis=1)
    selfexp_f = np.exp(selfdot_f / TAU)
    selfdot_c = np.sum(zca64 * zca64, axis=1)
    selfexp_c = np.exp(selfdot_c / TAU)

    # fg/bg loss
    npos_fg = n_fg - 1
    if npos_fg > 0:
        denom = stats[:, 0] - selfexp_f
        numer = stats[:, 1] - selfexp_f
        li = -np.log((numer + EPS) / (denom + EPS))
        loss_fg = np.sum(li * w_a) / (np.sum(w_a) + EPS)
    else:
        loss_fg = 0.0  # num=0, den=EPS -> 0

    # class supcon loss
    lab_valid = labels_s[:n_valid]
    cnt = np.bincount(lab_valid, minlength=21)
    S = np.zeros((21, DC), dtype=np.float64)
    np.add.at(S, lab_valid, zcn[:n_valid].astype(np.float64))
    c_a = labels_s[:n_A]
    n_pos = (cnt[c_a] - 1).astype(np.float64)
    D = stats[:, 2] - selfexp_c
    denom_log = np.log(np.maximum(D, 1e-300))
    sum_pos = (np.einsum("nd,nd->n", zca64, S[c_a]) - selfdot_c) / TAU
    li_c = -(sum_pos - n_pos * denom_log) / np.maximum(n_pos, 1.0)
    valid_c = n_pos > 0
    num2 = np.sum(np.where(valid_c, li_c * w_a, 0.0))
    den2 = np.sum(np.where(valid_c, w_a, 0.0))
    loss_cls = num2 / (den2 + EPS12)

    return np.stack([loss_fg, loss_cls]).astype(np.float32)


# revision 22
# speedup vs baseline: 1.1087x; 1.0538x over previous
"""Trainium2 Bass kernel for nn_MultiHeadContrastive (two-head contrastive loss).

Strategy (8 NeuronCores, two SPMD launches, no collectives):

  Launch 1 (MLP): rows of roi_feats are sorted by group
  (anchor / fg-low-iou / bg / ignore) on the host and sharded contiguously,
  1024 rows per core.  Each core computes both projection heads for its rows
  (transposed layout zT = [d, rows], fp32) via TensorE and returns the raw
  (pre-normalization) embeddings.

  Host: gathers the 8 z shards, L2-normalizes rows in float64, casts fp32.

  Launch 2 (SIM): every core receives the full normalized key matrices
  zT_fg [64, 8192], zT_cls [128, 8192] plus its private slice of anchor
  columns.  For each 128-anchor block it computes sim^T = anchors x keys via
  TensorE into PSUM (raw dot products), then ScalarE evaluates
  exp(dot / TAU) in place with accum_out producing per-anchor row sums per
  key range.  Because rows were sorted, the three masked sums the losses
  need (all keys / fg keys / non-ignored keys) are plain prefix-range sums,
  so no mask tensors and no second pass over the N^2 matrix exist at all.
  Anchors are restricted to rows with label>0, not ignored, and iou>0.5 —
  every other row contributes exactly zero to the weighted losses.

  Host: subtracts the self-similarity terms, computes the class-positive
  term of the SupCon loss from per-class sums of z (an O(N*D) computation),
  applies logs/weights in float64, and returns the 2-element loss vector.
"""

import math
import os

import numpy as np

import concourse.bass as bass
import concourse.bacc as bacc
import concourse.mybir as mybir
import concourse.tile as tile
from concourse.bass_utils import run_bass_kernel_spmd

N_CORES = 8
N, C = 8192, 1024
HID, DF, DC = 256, 64, 128
TAU = 0.2
EPS = 1e-8
EPS12 = 1e-12
IOU_THRESHOLD = 0.5

F32 = mybir.dt.float32
F32R = mybir.dt.float32r
ACT = mybir.ActivationFunctionType
AX = mybir.AxisListType

# Introspection for test.py: BassKernelResults of the two launches.
LAST_RESULTS = []


def _build_mlp_nc():
    """Launch 1: per-core MLP producing raw zT for both heads."""
    R = N // N_CORES  # rows per core
    KC = C // 128     # feature chunks
    KH = HID // 128   # hidden chunks
    RB = 512          # moving free dim per matmul (fp32 limit)
    NR = R // RB

    nc = bacc.Bacc(trn_type="TRN2", num_devices=N_CORES, debug=False)
    xT = nc.dram_tensor("xT", [C, R], F32R, kind="ExternalInput")
    w1fT = nc.dram_tensor("w1fT", [C, HID], F32R, kind="ExternalInput")
    w2fT = nc.dram_tensor("w2fT", [HID, DF], F32R, kind="ExternalInput")
    w1cT = nc.dram_tensor("w1cT", [C, HID], F32R, kind="ExternalInput")
    w2cT = nc.dram_tensor("w2cT", [HID, DC], F32R, kind="ExternalInput")
    b1f = nc.dram_tensor("b1f", [HID, 1], F32, kind="ExternalInput")
    b2f = nc.dram_tensor("b2f", [DF, 1], F32, kind="ExternalInput")
    b1c = nc.dram_tensor("b1c", [HID, 1], F32, kind="ExternalInput")
    b2c = nc.dram_tensor("b2c", [DC, 1], F32, kind="ExternalInput")
    zf = nc.dram_tensor("zf", [DF, R], F32, kind="ExternalOutput")
    zc = nc.dram_tensor("zc", [DC, R], F32, kind="ExternalOutput")

    with tile.TileContext(nc) as tc:
        with (
            tc.tile_pool(name="cst", bufs=1) as cst,
            tc.tile_pool(name="hb", bufs=2) as hb,
            tc.tile_pool(name="zb", bufs=2) as zb,
            tc.tile_pool(name="ps", bufs=1, space="PSUM") as ps,
        ):
            # per-chunk tiles so matmuls on chunk k only wait for chunk k's
            # DMA; interleave x and w1 chunk loads so the k-th chain step has
            # both operands as early as possible.
            heads = (
                ("f", w1fT, w2fT, b1f, b2f, DF, zf),
                ("c", w1cT, w2cT, b1c, b2c, DC, zc),
            )
            xt_t = []
            w1_t = {"f": [], "c": []}
            for k in range(KC):
                t = cst.tile([128, R], F32R, tag=f"xt{k}")
                nc.sync.dma_start(out=t[:, :], in_=xT[k * 128:(k + 1) * 128, :])
                xt_t.append(t)
                for hname, w1d, *_ in heads:
                    tw = cst.tile([128, HID], F32R, tag=f"w1{hname}{k}")
                    nc.sync.dma_start(out=tw[:, :], in_=w1d[k * 128:(k + 1) * 128, :])
                    w1_t[hname].append(tw)

            for hi, (hname, w1d, w2d, b1d, b2d, d, zout) in enumerate(heads):
                w2t = cst.tile([128, KH, d], F32R, tag=f"w2{hname}")
                for h in range(KH):
                    nc.sync.dma_start(out=w2t[:, h, :], in_=w2d[h * 128:(h + 1) * 128, :])
                b1t = cst.tile([128, KH], F32, tag=f"b1{hname}")
                for h in range(KH):
                    nc.sync.dma_start(out=b1t[:, h:h + 1], in_=b1d[h * 128:(h + 1) * 128, :])
                b2t = cst.tile([d, 1], F32, tag=f"b2{hname}")
                nc.sync.dma_start(out=b2t[:, :], in_=b2d[:, :])

                hsb = hb.tile([128, KH, R], F32R, tag=f"h{hname}")
                # all four (h, r) accumulation chains advance together as each
                # xT chunk lands, so PE finishes ~right after the last chunk
                hps = {}
                for h in range(KH):
                    for r in range(NR):
                        pidx = hi * 4 + h * NR + r
                        hps[(h, r)] = ps.tile(
                            [128, RB], F32, tag=f"p{pidx}", name=f"hp{pidx}"
                        )
                for k in range(KC):
                    for (h, r), hp in hps.items():
                        nc.tensor.matmul(
                            out=hp[:, :],
                            lhsT=w1_t[hname][k][:, h * 128:(h + 1) * 128],
                            rhs=xt_t[k][:, r * RB:(r + 1) * RB],
                            start=(k == 0),
                            stop=(k == KC - 1),
                        )
                for r in range(NR):
                    for h in range(KH):
                        # hT = relu(w1 @ xT + b1) fused on DVE (also rounds
                        # to fp32r for the next matmul); b1 is per-partition.
                        nc.vector.tensor_scalar(
                            out=hsb[:, h, r * RB:(r + 1) * RB],
                            in0=hps[(h, r)][:, :],
                            scalar1=b1t[:, h:h + 1],
                            scalar2=0.0,
                            op0=mybir.AluOpType.add,
                            op1=mybir.AluOpType.max,
                        )
                    # reuse the bank of the (h0, r) chain this head just
                    # drained via its relu — PSUM stays within 8 banks
                    zp = ps.tile([128, RB], F32, tag=f"p{hi * 4 + r}", name=f"zp{hi}{r}")
                    for h in range(KH):
                        nc.tensor.matmul(
                            out=zp[:d, :],
                            lhsT=w2t[:, h, :],
                            rhs=hsb[:, h, r * RB:(r + 1) * RB],
                            start=(h == 0),
                            stop=(h == KH - 1),
                        )
                    zt = zb.tile([d, RB], F32, tag=f"z{hname}")
                    nc.scalar.activation(
                        out=zt[:, :],
                        in_=zp[:d, :],
                        func=ACT.Identity,
                        bias=b2t[:, 0:1],
                        scale=1.0,
                    )
                    nc.sync.dma_start(out=zout[:, r * RB:(r + 1) * RB], in_=zt[:, :])
    nc.compile()
    return nc


def _build_sim_nc(n_fg, n_valid, nblk):
    """Launch 2: per-anchor-block sim matmuls + fused exp/prefix-range sums.

    Returns (nc, numer_cols, nfgcols, ngc): stats output columns are
      0: sum_{all keys} exp(sim/TAU)
      1: sum_{keys < n_fg} exp(sim/TAU)
      2: sum_{keys < n_valid} exp(sim/TAU)
    (all including the anchor's self term, subtracted on the host).
    """
    A = nblk * 128
    G = 2048
    NGF = N // G
    NGC = (n_valid + G - 1) // G

    nc = bacc.Bacc(trn_type="TRN2", num_devices=N_CORES, debug=False)
    zfk = nc.dram_tensor("zfk", [DF, N], F32R, kind="ExternalInput")
    zck = nc.dram_tensor("zck", [DC, N], F32R, kind="ExternalInput")
    zfa = nc.dram_tensor("zfa", [DF, A], F32R, kind="ExternalInput")
    zca = nc.dram_tensor("zca", [DC, A], F32R, kind="ExternalInput")
    stats = nc.dram_tensor("stats", [nblk, 128, 3], F32, kind="ExternalOutput")

    # fg-head exp/accum pieces: split each 2048-key group at the n_fg
    # boundary so masked sums become plain column-range selections.
    fg_pieces = []  # (group, c0, c1, col)
    col = 0
    numer_cols = 0
    for g in range(NGF):
        lo, hi = g * G, (g + 1) * G
        cuts = [lo, n_fg, hi] if lo < n_fg < hi else [lo, hi]
        for a0, a1 in zip(cuts[:-1], cuts[1:]):
            fg_pieces.append((g, a0 - lo, a1 - lo, col))
            if a1 <= n_fg:
                numer_cols = col + 1
            col += 1
    nfgcols = col

    with tile.TileContext(nc) as tc:
        with (
            tc.tile_pool(name="keys", bufs=1) as keys,
            tc.tile_pool(name="anch", bufs=1) as anch,
            tc.tile_pool(name="st", bufs=3) as st,
            tc.tile_pool(name="ps", bufs=2, space="PSUM") as ps,
        ):
            # anchors first: the very first matmul needs them
            zfa_t = anch.tile([DF, A], F32R, tag="zfa")
            nc.sync.dma_start(out=zfa_t[:, :], in_=zfa[:, :])
            zca_t = anch.tile([DC, A], F32R, tag="zca")
            nc.sync.dma_start(out=zca_t[:, :], in_=zca[:, :])
            # warm up the ACT exp table load while DMAs stream
            wu = st.tile([1, 8], F32, tag="wu")
            nc.vector.memset(wu[:, :], 0.0)
            nc.scalar.activation(out=wu[:, :], in_=wu[:, :], func=ACT.Exp, scale=1.0)
            zfk_t = []
            for g in range(NGF):
                t = keys.tile([DF, G], F32R, tag=f"zfk{g}")
                nc.sync.dma_start(out=t[:, :], in_=zfk[:, g * G:(g + 1) * G])
                zfk_t.append(t)
            zck_t = []
            for g in range(NGC):
                # load the full group (cols past n_valid are real rows too);
                # only the exp/accum below is range-restricted
                t = keys.tile([DC, G], F32R, tag=f"zck{g}")
                nc.sync.dma_start(out=t[:, :], in_=zck[:, g * G:(g + 1) * G])
                zck_t.append(t)

            for ab in range(nblk):
                lf = zfa_t[:, ab * 128:(ab + 1) * 128]
                lc = zca_t[:, ab * 128:(ab + 1) * 128]
                sf = st.tile([128, nfgcols], F32, tag="sf")
                sc = st.tile([128, NGC], F32, tag="sc")
                for g in range(NGF):
                    p = ps.tile([128, G], F32, tag="ps")
                    for kk in range(G // 512):
                        nc.tensor.matmul(
                            out=p[:, kk * 512:(kk + 1) * 512],
                            lhsT=lf,
                            rhs=zfk_t[g][:, kk * 512:(kk + 1) * 512],
                            start=True,
                            stop=True,
                        )
                    for gg, c0, c1, pcol in fg_pieces:
                        if gg != g:
                            continue
                        nc.scalar.activation(
                            out=p[:, c0:c1],
                            in_=p[:, c0:c1],
                            func=ACT.Exp,
                            scale=1.0 / TAU,
                            accum_out=sf[:, pcol:pcol + 1],
                        )
                for g in range(NGC):
                    klim = min(G, n_valid - g * G)
                    p = ps.tile([128, G], F32, tag="ps")
                    # full-width matmuls (fp32r needs large even free dims);
                    # only [0:klim] is exp'd/accumulated below
                    for kk in range(G // 512):
                        if kk * 512 >= klim:
                            break
                        nc.tensor.matmul(
                            out=p[:, kk * 512:(kk + 1) * 512],
                            lhsT=lc,
                            rhs=zck_t[g][:, kk * 512:(kk + 1) * 512],
                            start=True,
                            stop=True,
                        )
                    nc.scalar.activation(
                        out=p[:, 0:klim],
                        in_=p[:, 0:klim],
                        func=ACT.Exp,
                        scale=1.0 / TAU,
                        accum_out=sc[:, g:g + 1],
                    )
                o3 = st.tile([128, 3], F32, tag="o3")
                nc.vector.reduce_sum(out=o3[:, 0:1], in_=sf[:, 0:nfgcols], axis=AX.X)
                nc.vector.reduce_sum(out=o3[:, 1:2], in_=sf[:, 0:numer_cols], axis=AX.X)
                nc.vector.reduce_sum(out=o3[:, 2:3], in_=sc[:, 0:NGC], axis=AX.X)
                nc.sync.dma_start(out=stats[ab, :, :], in_=o3[:, :])
    nc.compile()
    return nc


LAST_TIMES = []


def _run(nc, in_maps, out_names):
    import time as _time

    if os.environ.get("CC_BASS_SIM") == "1":
        from concourse import bass_interp

        results = []
        for m in range(N_CORES):
            sim = bass_interp.CoreSim(nc, core_id=m)
            for k, v in in_maps[m].items():
                sim.tensor(k)[:] = v
            if nc.partition_id_tensor is not None:
                sim.tensor(nc.partition_id_tensor.name)[:] = np.array(
                    [[m]], dtype=np.uint32
                )
            sim.simulate()
            results.append(
                {name: np.array(sim.mem_tensor(name)) for name in out_names}
            )
        return results
    t0 = _time.monotonic()
    res = run_bass_kernel_spmd(nc, in_maps, core_ids=list(range(N_CORES)))
    LAST_TIMES.append(_time.monotonic() - t0)
    LAST_RESULTS.append(res)
    return res.results


def kernel(**inputs):
    global LAST_RESULTS, LAST_TIMES
    LAST_RESULTS = []
    LAST_TIMES = []

    roi = np.ascontiguousarray(np.asarray(inputs["roi_feats"], dtype=np.float32))
    labels = np.asarray(inputs["labels"]).astype(np.int64)
    ious = np.asarray(inputs["ious"], dtype=np.float32)
    w1f = np.asarray(inputs["w1f"], dtype=np.float32)
    b1f = np.asarray(inputs["b1f"], dtype=np.float32)
    w2f = np.asarray(inputs["w2f"], dtype=np.float32)
    b2f = np.asarray(inputs["b2f"], dtype=np.float32)
    w1c = np.asarray(inputs["w1c"], dtype=np.float32)
    b1c = np.asarray(inputs["b1c"], dtype=np.float32)
    w2c = np.asarray(inputs["w2c"], dtype=np.float32)
    b2c = np.asarray(inputs["b2c"], dtype=np.float32)
    assert roi.shape == (N, C)

    ign = labels == -1
    fg = (labels > 0) & ~ign
    bg = (labels == 0) & ~ign
    anc = fg & (ious > IOU_THRESHOLD)

    perm = np.concatenate(
        [
            np.where(anc)[0],
            np.where(fg & ~anc)[0],
            np.where(bg)[0],
            np.where(ign)[0],
        ]
    )
    n_A = int(anc.sum())
    n_fg = int(fg.sum())
    n_valid = n_fg + int(bg.sum())

    if n_A == 0:
        return np.zeros(2, dtype=np.float32)

    x_s = roi[perm]
    labels_s = labels[perm]
    ious_s = ious[perm].astype(np.float64)

    # ---------------- launch 1: MLP ----------------
    nc1 = _build_mlp_nc()
    xT = np.ascontiguousarray(x_s.T)  # [C, N]
    R = N // N_CORES
    shared1 = {
        "w1fT": np.ascontiguousarray(w1f.T),
        "w2fT": np.ascontiguousarray(w2f.T),
        "w1cT": np.ascontiguousarray(w1c.T),
        "w2cT": np.ascontiguousarray(w2c.T),
        "b1f": b1f.reshape(HID, 1).copy(),
        "b2f": b2f.reshape(DF, 1).copy(),
        "b1c": b1c.reshape(HID, 1).copy(),
        "b2c": b2c.reshape(DC, 1).copy(),
    }
    in_maps1 = [
        {"xT": np.ascontiguousarray(xT[:, m * R:(m + 1) * R]), **shared1}
        for m in range(N_CORES)
    ]
    res1 = _run(nc1, in_maps1, ["zf", "zc"])

    zfT_raw = np.concatenate([r["zf"] for r in res1], axis=1)  # [DF, N]
    zcT_raw = np.concatenate([r["zc"] for r in res1], axis=1)  # [DC, N]

    # ---------------- host: normalize in float64, cast fp32 ----------------
    def _normalize(zT_raw):
        z = zT_raw.T.astype(np.float64)  # [N, d]
        nrm = np.sqrt(np.sum(z * z, axis=1, keepdims=True))
        zn = z / np.maximum(nrm, EPS)
        return zn.astype(np.float32)

    zfn = _normalize(zfT_raw)  # [N, DF] fp32, sorted order
    zcn = _normalize(zcT_raw)  # [N, DC]

    # ---------------- launch 2: sims ----------------
    nblk = max(1, math.ceil(math.ceil(n_A / N_CORES) / 128))
    A_pc = nblk * 128
    nc2 = _build_sim_nc(n_fg, n_valid, nblk)

    zfkT = np.ascontiguousarray(zfn.T)  # [DF, N]
    zckT = np.ascontiguousarray(zcn.T)  # [DC, N]
    in_maps2 = []
    for m in range(N_CORES):
        idx = np.minimum(np.arange(m * A_pc, (m + 1) * A_pc), n_A - 1)
        in_maps2.append(
            {
                "zfk": zfkT,
                "zck": zckT,
                "zfa": np.ascontiguousarray(zfkT[:, idx]),
                "zca": np.ascontiguousarray(zckT[:, idx]),
            }
        )
    res2 = _run(nc2, in_maps2, ["stats"])

    # slot s of the concatenated stats covers anchor s; drop padded slots
    stats = np.concatenate([r["stats"].reshape(A_pc, 3) for r in res2], axis=0)
    stats = stats[np.arange(N_CORES * A_pc) < n_A].astype(np.float64)  # [n_A, 3]

    # ---------------- host: final losses in float64 ----------------
    zfa64 = zfn[:n_A].astype(np.float64)
    zca64 = zcn[:n_A].astype(np.float64)
    w_a = ious_s[:n_A]

    selfdot_f = np.sum(zfa64 * zfa64, axis=1)
    selfexp_f = np.exp(selfdot_f / TAU)
    selfdot_c = np.sum(zca64 * zca64, axis=1)
    selfexp_c = np.exp(selfdot_c / TAU)

    # fg/bg loss
    npos_fg = n_fg - 1
    if npos_fg > 0:
        denom = stats[:, 0] - selfexp_f
        numer = stats[:, 1] - selfexp_f
        li = -np.log((numer + EPS) / (denom + EPS))
        loss_fg = np.sum(li * w_a) / (np.sum(w_a) + EPS)
    else:
        loss_fg = 0.0  # num=0, den=EPS -> 0

    # class supcon loss
    lab_valid = labels_s[:n_valid]
    cnt = np.bincount(lab_valid, minlength=21)
    S = np.zeros((21, DC), dtype=np.float64)
    np.add.at(S, lab_valid, zcn[:n_valid].astype(np.float64))
    c_a = labels_s[:n_A]
    n_pos = (cnt[c_a] - 1).astype(np.float64)
    D = stats[:, 2] - selfexp_c
    denom_log = np.log(np.maximum(D, 1e-300))
    sum_pos = (np.einsum("nd,nd->n", zca64, S[c_a]) - selfdot_c) / TAU
    li_c = -(sum_pos - n_pos * denom_log) / np.maximum(n_pos, 1.0)
    valid_c = n_pos > 0
    num2 = np.sum(np.where(valid_c, li_c * w_a, 0.0))
    den2 = np.sum(np.where(valid_c, w_a, 0.0))
    loss_cls = num2 / (den2 + EPS12)

    return np.stack([loss_fg, loss_cls]).astype(np.float32)
